# revision 14
# baseline (speedup 1.0000x reference)
"""Trainium2 Bass kernel for CustomMultiheadAttention.

Problem: B=2, S=2048, E=1024, H=16 heads, D=64. Returns (out, attn_weights).

Sharding (8 cores): core c = b*4 + hg handles batch b and head-group hg
(4 heads). Megatron-style tensor parallel on the QKV/out projections.

Per-core device kernel (all layouts chosen so no on-device transposes are
needed; host pre-transposes/casts inputs):
  - QT/KT = [hd=256, s=2048] (feature-on-partition) via W.T @ x.T matmuls
  - V     = [s=2048, hd] natural layout, stored per k-tile with a ones
            column appended per head (V_aug [128, kt, h, 65])
  - per (head, q-half): scores^T [k, q] tiles -> exp on ScalarE (bf16)
  - PV matmul with V_aug gives attn_out^T [d, q] AND softmax denominators
    (row 64) in one accumulation chain
  - ones-outer-product matmul broadcasts the sums row to all partitions,
    then full-partition DVE reciprocal / normalize (bf16 2x mode)
  - attn_weights written TRANSPOSED per head ([h, k, q]) in bf16 (values
    are bf16 products already); host transposes + upcasts
  - out projection contracts per-head (K=64) over attn_out^T tiles
Host gathers: out[b] = sum of 4 partial outs + bo; attn_weights = P^T.T
"""

import numpy as np
import ml_dtypes

import concourse.bass as bass
import concourse.mybir as mybir
import concourse.tile as tile
from concourse import bacc
from concourse.bass_utils import run_bass_kernel_spmd

F32 = mybir.dt.float32
BF16 = mybir.dt.bfloat16
AF = mybir.ActivationFunctionType
ALU = mybir.AluOpType

B, S, E, H, D = 2, 2048, 1024, 16, 64
NCORES = 8
HPC = 4          # heads per core
HD = HPC * D     # 256, per-core projection width
P = 128
KT = S // P      # 16 k-tiles
QH = 2           # q halves
QB = S // QH     # 1024 per half
KO = E // P      # 8 contraction tiles for projections

_BUILT = {}

_IN_SPECS = (
    ("xqT", [E, S], BF16), ("xkT", [E, S], BF16), ("xvT", [E, S], BF16),
    ("wqT", [E, HD], BF16), ("wkT", [E, HD], BF16), ("wvT", [E, HD], BF16),
    ("woT", [D, HPC, E], BF16), ("bqv", [2, HD], F32), ("bvv", [HD], F32),
)


def _emit_prologue(nc, tc, T, consts, qk):
    """Load persistent weights/biases, allocate persistent activation tiles."""
    t = {}
    t["wq_sb"] = consts.tile([P, KO, HD], BF16, tag="wq", name="wq_sb")
    nc.sync.dma_start(t["wq_sb"][:], T["wqT"].rearrange("(ko p) m -> p ko m", p=P))
    t["wk_sb"] = consts.tile([P, KO, HD], BF16, tag="wk", name="wk_sb")
    nc.sync.dma_start(t["wk_sb"][:], T["wkT"].rearrange("(ko p) m -> p ko m", p=P))
    t["wv_sb"] = consts.tile([P, KO, HD], BF16, tag="wv", name="wv_sb")
    nc.sync.dma_start(t["wv_sb"][:], T["wvT"].rearrange("(ko p) m -> p ko m", p=P))
    t["wo_sb"] = consts.tile([D, HPC, E], BF16, tag="wo", name="wo_sb")
    nc.sync.dma_start(t["wo_sb"][:], T["woT"][:])
    t["bqk_sb"] = consts.tile([P, 4], F32, tag="bqk", name="bqk_sb")
    nc.sync.dma_start(t["bqk_sb"][:], T["bqv"].rearrange("t (m p) -> p (t m)", p=P))
    t["ones_sb"] = consts.tile([P, P], BF16, tag="ones", name="ones_sb")
    nc.vector.memset(t["ones_sb"][:], 1.0)
    t["bv_sb"] = consts.tile([P, HD], F32, tag="bv", name="bv_sb")
    bv_ap = T["bvv"][:]
    nc.gpsimd.dma_start(
        out=t["bv_sb"][:],
        in_=bass.AP(
            tensor=bv_ap.tensor,
            offset=bv_ap.offset,
            ap=[[0, P]] + [list(d) for d in bv_ap.ap],
        ),
    )
    t["qt_sb"] = [qk.tile([P, S], BF16, tag=f"qt{j}", name=f"qt{j}") for j in range(2)]
    t["kt_sb"] = [qk.tile([P, S], BF16, tag=f"kt{j}", name=f"kt{j}") for j in range(2)]
    t["vaug"] = qk.tile([P, KT, HPC, D + 1], BF16, tag="vaug", name="vaug")
    t["ao_sb"] = [qk.tile([D, S], BF16, tag=f"ao{h}", name=f"ao{h}") for h in range(HPC)]
    return t


def _emit_body(nc, tc, T, t, pools):
    xpool, epool, rpool, ppool, opool, psA, psB = pools
    qt_sb, kt_sb, vaug, ao_sb = t["qt_sb"], t["kt_sb"], t["vaug"], t["ao_sb"]

    # ones column at [..., 64]; V writes below leave it intact
    nc.vector.memset(vaug[:], 1.0)

    # ---- phase 1: projections ----
    xq_t = T["xqT"].rearrange("(ko p) s -> p ko s", p=P)
    xk_t = T["xkT"].rearrange("(ko p) s -> p ko s", p=P)
    xv_t = T["xvT"].rearrange("(ko p) s -> p ko s", p=P)
    for sb in range(4):  # s blocks of 512
        s0 = sb * 512
        x_sbs = []
        for nm, xt in (("xq", xq_t), ("xk", xk_t), ("xv", xv_t)):
            xtile = xpool.tile([P, KO, 512], BF16, tag="x", name=f"x_{nm}{sb}")
            nc.sync.dma_start(xtile[:], xt[:, :, s0 : s0 + 512])
            x_sbs.append(xtile)
        xq_sb, xk_sb, xv_sb = x_sbs
        for j in range(2):  # hd tiles
            for ti, (w_sb, x_sb, dstl) in enumerate(
                ((t["wq_sb"], xq_sb, qt_sb), (t["wk_sb"], xk_sb, kt_sb))
            ):
                ps = psB.tile([P, 512], F32, tag="ps_mm", bufs=1, name="ps_prj")
                for ko in range(KO):
                    nc.tensor.matmul(
                        ps[:],
                        lhsT=w_sb[:, ko, j * P : (j + 1) * P],
                        rhs=x_sb[:, ko, :],
                        start=(ko == 0),
                        stop=(ko == KO - 1),
                    )
                nc.vector.tensor_scalar_add(
                    dstl[j][:, s0 : s0 + 512],
                    ps[:],
                    t["bqk_sb"][:, 2 * ti + j : 2 * ti + j + 1],
                )
        # V: natural layout, into vaug
        for st4 in range(4):
            st = sb * 4 + st4
            ps = psB.tile([P, 512], F32, tag="ps_mm", bufs=1, name="ps_v")
            for ko in range(KO):
                nc.tensor.matmul(
                    ps[:, :HD],
                    lhsT=xv_sb[:, ko, st4 * P : (st4 + 1) * P],
                    rhs=t["wv_sb"][:, ko, :],
                    start=(ko == 0),
                    stop=(ko == KO - 1),
                )
            nc.vector.tensor_tensor(
                out=vaug[:, st, :, 0:D],
                in0=ps[:, :HD].rearrange("p (h d) -> p h d", d=D),
                in1=t["bv_sb"][:].rearrange("p (h d) -> p h d", d=D),
                op=ALU.add,
            )

    # ---- phase 2: attention per (head, q-half) ----
    for h in range(HPC):
        j, po = h // 2, (h % 2) * D
        for qh in range(QH):
            q0 = qh * QB
            e_sb = epool.tile([P, KT, QB], BF16, tag="e", name="e_sb")
            for kt in range(KT):
                ps_s = psA.tile([P, QB], F32, tag="ps_s", bufs=2, name="ps_s")
                for qq in range(2):
                    nc.tensor.matmul(
                        ps_s[:, qq * 512 : (qq + 1) * 512],
                        lhsT=kt_sb[j][po : po + D, kt * P : (kt + 1) * P],
                        rhs=qt_sb[j][po : po + D, q0 + qq * 512 : q0 + (qq + 1) * 512],
                        start=True,
                        stop=True,
                    )
                nc.scalar.activation(e_sb[:, kt, :], ps_s[:], AF.Exp)
            # PV + row sums; ones-outer-product broadcast of sums; normalize
            sums_b = rpool.tile([P, QB], BF16, tag="sums_b", name="sums_b")
            rec_f = rpool.tile([P, QB], F32, tag="rec_f", name="rec_f")
            rec_b = rpool.tile([P, QB], BF16, tag="rec_b", name="rec_b")
            for qq in range(2):
                qs = slice(qq * 512, (qq + 1) * 512)
                ps_av = psB.tile([P, 512], F32, tag="ps_av", bufs=2, name="ps_av")
                for kt in range(KT):
                    nc.tensor.matmul(
                        ps_av[0 : D + 1, :],
                        lhsT=vaug[:, kt, h, :],
                        rhs=e_sb[:, kt, qs],
                        start=(kt == 0),
                        stop=(kt == KT - 1),
                    )
                nc.vector.tensor_copy(sums_b[D : D + 1, qs], ps_av[D : D + 1, :])
                ps_bc = psA.tile([P, 512], F32, tag="ps_bc", bufs=1, name="ps_bc")
                nc.tensor.matmul(
                    ps_bc[:],
                    lhsT=t["ones_sb"][D : D + 1, :],
                    rhs=sums_b[D : D + 1, qs],
                    start=True,
                    stop=True,
                )
                nc.vector.reciprocal(rec_f[:, qs], ps_bc[:])
                nc.vector.tensor_copy(rec_b[:, qs], rec_f[:, qs])
                nc.vector.tensor_mul(
                    ao_sb[h][:, q0 + qq * 512 : q0 + (qq + 1) * 512],
                    ps_av[0:D, :],
                    rec_b[0:D, qs],
                )
            for ktp in range(KT // 2):
                p_sb = ppool.tile([P, 2, QB], BF16, tag="p", name="p_sb")
                for u in range(2):
                    nc.vector.tensor_mul(
                        p_sb[:, u, :], e_sb[:, ktp * 2 + u, :], rec_b[:]
                    )
                nc.sync.dma_start(
                    out=T["pT"][h, ktp * 256 : (ktp + 1) * 256, q0 : q0 + QB].rearrange(
                        "(u p) q -> p u q", p=P
                    ),
                    in_=p_sb[:],
                )

    # ---- phase 3: out projection ----
    for st in range(S // P):
        o_sb = opool.tile([P, E], F32, tag="o", name="o_sb")
        for eb in range(2):
            ps_o = psB.tile([P, 512], F32, tag="ps_mm", bufs=1, name="ps_o")
            for h in range(HPC):
                nc.tensor.matmul(
                    ps_o[:],
                    lhsT=ao_sb[h][:, st * P : (st + 1) * P],
                    rhs=t["wo_sb"][:, h, eb * 512 : (eb + 1) * 512],
                    start=(h == 0),
                    stop=(h == HPC - 1),
                )
            nc.scalar.copy(o_sb[:, eb * 512 : (eb + 1) * 512], ps_o[:])
        nc.sync.dma_start(T["outp"][st * P : (st + 1) * P, :], o_sb[:])


def _build(timing_reps=0):
    """Build and finalize the SPMD Bass module.

    timing_reps=0: real kernel (external I/O).
    timing_reps>0: timing kernel — internal DRAM I/O (content irrelevant),
    body repeated timing_reps times in a hardware loop, tiny external token
    output so the executable has stable minimal transfer size.
    """
    nc = bacc.Bacc("TRN2", target_bir_lowering=False)
    timing = timing_reps > 0
    kind_in = "Internal" if timing else "ExternalInput"
    kind_out = "Internal" if timing else "ExternalOutput"
    T = {}
    for nm, shape, dt in _IN_SPECS:
        T[nm] = nc.dram_tensor(nm, shape, dt, kind=kind_in)
    T["pT"] = nc.dram_tensor("pT", [HPC, S, S], BF16, kind=kind_out)
    T["outp"] = nc.dram_tensor("outp", [S, E], F32, kind=kind_out)
    if timing:
        tok_in = nc.dram_tensor("tok_in", [P, 4], F32, kind="ExternalInput")
        tok_out = nc.dram_tensor("tok_out", [P, 4], F32, kind="ExternalOutput")

    with tile.TileContext(nc) as tc:
        with (
            tc.tile_pool(name="consts", bufs=1) as consts,
            tc.tile_pool(name="xpool", bufs=4) as xpool,
            tc.tile_pool(name="qk", bufs=1) as qk,
            tc.tile_pool(name="epool", bufs=2) as epool,
            tc.tile_pool(name="rpool", bufs=1) as rpool,
            tc.tile_pool(name="ppool", bufs=3) as ppool,
            tc.tile_pool(name="opool", bufs=2) as opool,
            tc.tile_pool(name="psA", bufs=1, space="PSUM") as psA,
            tc.tile_pool(name="psB", bufs=1, space="PSUM") as psB,
        ):
            t = _emit_prologue(nc, tc, T, consts, qk)
            pools = (xpool, epool, rpool, ppool, opool, psA, psB)
            if timing:
                tok = consts.tile([P, 4], F32, tag="tok", name="tok")
                nc.sync.dma_start(tok[:], tok_in[:])
                with tc.For_i(0, timing_reps, 1) as _i:
                    _emit_body(nc, tc, T, t, pools)
                nc.sync.dma_start(tok_out[:], tok[:])
            else:
                _emit_body(nc, tc, T, t, pools)

    nc.finalize()
    return nc


def _host_prep(query, key, value, Wq, bq, Wk, bk, Wv, bv, Wo, bo):
    """Build per-core input maps. Core c = b*4 + hg."""
    scale = 1.0 / np.sqrt(D)
    bf = ml_dtypes.bfloat16
    in_maps = []
    xTs = {}
    for b in range(B):
        xTs[b] = (
            np.ascontiguousarray(query[b].T).astype(bf),
            np.ascontiguousarray(key[b].T).astype(bf),
            np.ascontiguousarray(value[b].T).astype(bf),
        )
    for c in range(NCORES):
        b, hg = c // 4, c % 4
        hs = slice(hg * HD, (hg + 1) * HD)
        xqT, xkT, xvT = xTs[b]
        wqT = np.ascontiguousarray((Wq[hs, :] * scale).T).astype(bf)
        wkT = np.ascontiguousarray(Wk[hs, :].T).astype(bf)
        wvT = np.ascontiguousarray(Wv[hs, :].T).astype(bf)
        woT = np.ascontiguousarray(
            Wo[:, hs].T.reshape(HPC, D, E).transpose(1, 0, 2)
        ).astype(bf)
        bqv = np.stack([bq[hs] * scale, bk[hs]]).astype(np.float32)
        bvv = bv[hs].astype(np.float32)
        in_maps.append(
            {
                "xqT": xqT, "xkT": xkT, "xvT": xvT,
                "wqT": wqT, "wkT": wkT, "wvT": wvT, "woT": woT,
                "bqv": np.ascontiguousarray(bqv), "bvv": np.ascontiguousarray(bvv),
            }
        )
    return in_maps


def run(query, key, value, Wq, bq, Wk, bk, Wv, bv, Wo, bo, trace=False):
    if "nc" not in _BUILT:
        _BUILT["nc"] = _build()
    nc = _BUILT["nc"]
    in_maps = _host_prep(query, key, value, Wq, bq, Wk, bk, Wv, bv, Wo, bo)
    res = run_bass_kernel_spmd(nc, in_maps, list(range(NCORES)), trace=trace)

    out = np.empty((B, S, E), np.float32)
    attn = np.empty((B, H, S, S), np.float32)
    for b in range(B):
        acc = None
        for hg in range(4):
            r = res.results[b * 4 + hg]
            acc = r["outp"] if acc is None else acc + r["outp"]
            pt = r["pT"]  # [4, S(k), S(q)] bf16
            for h in range(HPC):
                attn[b, hg * HPC + h] = pt[h].T
        out[b] = acc + bo[None, :].astype(np.float32)
    return (out, attn), res


def run_timing(reps=16, iters=5):
    """Estimate per-invocation device time via a repeated-body timing build."""
    import time
    key1, key2 = f"nc_t{reps}", "nc_t1"
    if key1 not in _BUILT:
        _BUILT[key1] = _build(timing_reps=reps)
    if key2 not in _BUILT:
        _BUILT[key2] = _build(timing_reps=1)
    tok = {"tok_in": np.zeros((P, 4), np.float32)}
    in_maps = [tok] * NCORES
    out = {}
    for k, nc in ((key1, _BUILT[key1]), (key2, _BUILT[key2])):
        run_bass_kernel_spmd(nc, in_maps, list(range(NCORES)))  # warm
        ts = []
        for _ in range(iters):
            t0 = time.perf_counter()
            run_bass_kernel_spmd(nc, in_maps, list(range(NCORES)))
            ts.append(time.perf_counter() - t0)
        out[k] = sorted(ts)
    med_R = out[key1][len(out[key1]) // 2]
    med_1 = out[key2][len(out[key2]) // 2]
    est = (med_R - med_1) / (reps - 1)
    return est, out


def kernel(query, key, value, Wq, bq, Wk, bk, Wv, bv, Wo, bo):
    (out, attn), _ = run(query, key, value, Wq, bq, Wk, bk, Wv, bv, Wo, bo)
    return out, attn
